# revision 59
# baseline (speedup 1.0000x reference)
"""Trainium2 Bass kernel for nn_NewAttention (analytic Gaussian sparse attention).

Math (per batch element b):
    v        = x[b] @ W_in.T                      # [L, E]
    per head h (P=128 cols of v):
        A_h  = softmax(-(j - c_h(i))^2 / 2)       # [L, L], analytic, banded
        att_h = A_h @ v_h                         # [L, P]
    out[b]   = concat_h(att_h) @ W_out.T          # [L, E]

Sharding: data-parallel over batch, one batch element per NeuronCore (8 cores).

Device strategy (per core):
  - 'first'/'last' heads (3/4) attend to a fixed key location for every query,
    so their output contribution is a single row vector r34[e] added to every
    output row. r34 only depends on 32 rows of x, so the HOST computes it
    exactly and ships it as a per-core [128, 8] bias table. Heads 3/4 then
    vanish from the device program entirely.
  - matmul1 (v = x @ W_in.T): fp8e4 DoubleRow with a dual-plane
    error-compensated split: x = xh + xl, 64*W = Wh + Wl (same scale for all
    planes), accumulate xh@Wh + xh@Wl + xl@Wh in one PSUM group; the dropped
    xl@Wl term and plane-residuals are ~0.2% — bf16-level accuracy at half
    the PE cost. The 1/64 descale rides the PSUM->SBUF copy for free.
  - attention: att^T_h = v_h.T @ A_h^T as banded bf16 matmuls: stationary =
    v 128x128 slices, moving = analytic A^T window blocks (host-precomputed
    exact softmax weights, truncated at |key-center| <= 4). All interior tiles
    share ONE shift-invariant [128, 136] window table; boundary tiles get
    exact renormalized tables. Windows split at PSUM-bank (512 col)
    boundaries and accumulate via per-element has_written bits. Attention
    column-groups are emitted inside the phase-1 tile loop as soon as their
    v tiles exist, so their PSUM->SBUF copies hide under phase-1 matmuls.
  - matmul2 computes out^T (feature-major): stationary = W_out^T slices,
    moving = att^T q-chunks, accumulated over the 6 banded heads — also in
    dual-plane fp8 DoubleRow (att planes are split on-device: hi = 8*att via
    one copy, lo = (8*att - hi) via one scalar_tensor_tensor). The r34 bias
    and the 1/(8*64) descale ride the PSUM->SBUF copy. Output leaves the
    device as bf16 out^T; the host transposes and upcasts.
  - PE p-state: dummy matmuls on a zeroed scratch tile run during the initial
    DMA fill so the clock ramp completes before real work arrives.
"""

import sys
import numpy as np

for _p in ("/opt/trn_rl_repo",):
    if _p not in sys.path:
        sys.path.insert(0, _p)

import concourse.bass as bass
import concourse.bacc as bacc
import concourse.mybir as mybir
from concourse import tile
from concourse import bass2jax as _b2j

# ---------------- problem constants (hardcoded per contract) ----------------
B = 8
L = 2048
E = 1024
H = 8
P = 128
SIGMA = 1.0
DISP = 1
NT = L // 128           # 16 seq tiles
R = 4                   # Gaussian taps kept each side of the center
WIN = 128 + 2 * R       # 136: per-tile attention query window
DT = mybir.dt.float32
BF = mybir.dt.bfloat16
F8 = mybir.dt.float8e4

GEMM1_FP8 = True    # dual-plane fp8e4 DoubleRow for v = x @ W_in.T
GEMM2_FP8 = True    # dual-plane fp8e4 DoubleRow for out^T = W_out @ att^T
W_SCALE = 64.0          # fp8 plane scale for W_in / W_out
ATT_SCALE = 8.0         # fp8 plane scale for att^T

BANDED_HEADS = [0, 1, 2, 5, 6, 7]   # center,left,right,center,left,right
NBH = len(BANDED_HEADS)
BI_TYPE = [0, 1, 2, 0, 1, 2]        # 0=center,1=left,2=right
TYPE_DISP = [0, -DISP, +DISP]
NQ4 = L // 512                      # 4 attention PSUM column groups
# emit attention (q4 group, head pair) at the end of phase-1 tile iteration i
ATT_EMIT_AT = {
    5: [(0, 0)], 6: [(0, 1)], 7: [(0, 2)],
    9: [(1, 0)], 10: [(1, 1)], 11: [(1, 2)],
    12: [(2, 0)], 13: [(2, 1)], 14: [(2, 2)],
    15: [(3, 0), (3, 1)],
}


def _g(x):
    return np.exp(-(np.asarray(x, dtype=np.float64) ** 2) / (2.0 * SIGMA**2))


def _attn_tables():
    """[128, 7*136] float: interior | first(t=0..2) | last(t=0..2) A^T blocks.

    interior[k, c] = g(k + R - c)/Zinf  (shift-invariant, shared by all head
    types and tiles 1..14; the head displacement only moves the window).
    first/last blocks are exact full-softmax values at the sequence edges.
    """
    tab = np.zeros((128, 7 * WIN), dtype=np.float64)
    zinf = _g(np.arange(-64, 65)).sum()
    k = np.arange(128)
    c = np.arange(WIN)
    delta = k[:, None] + R - c[None, :]
    tab[:, 0:WIN] = np.where(np.abs(delta) <= R, _g(delta) / zinf, 0.0)

    j = np.arange(L, dtype=np.float64)
    for t in range(3):
        d = TYPE_DISP[t]
        # first block: tile 0, queries q in [0, 132 - d)
        w = 132 - d
        q = np.arange(w, dtype=np.float64)
        logits = _g(j[None, :] - (q[:, None] + d))          # [w, L]
        A = logits / logits.sum(axis=1, keepdims=True)
        tab[:, (1 + t) * WIN : (1 + t) * WIN + w] = A[:, 0:128].T
        # last block: tile 15, queries q in [1916 - d, 2048)
        w = 132 + d
        q = np.arange(L - w, L, dtype=np.float64)
        logits = _g(j[None, :] - (q[:, None] + d))
        A = logits / logits.sum(axis=1, keepdims=True)
        tab[:, (4 + t) * WIN : (4 + t) * WIN + w] = A[:, L - 128 : L].T
    return tab


def _attn_pieces():
    """pieces[t][q4] = ordered [(tile i, col within 512-psum, width, table col)]."""
    pieces = [[[] for _ in range(NQ4)] for _ in range(3)]
    for t in range(3):
        d = TYPE_DISP[t]
        for i in range(NT):
            if i == 0:
                w0, w, base = 0, 132 - d, (1 + t) * WIN
            elif i == NT - 1:
                w0, w, base = 128 * i - R - d, 132 + d, (4 + t) * WIN
            else:
                w0, w, base = 128 * i - R - d, WIN, 0
            p0 = w0
            while p0 < w0 + w:
                q4 = p0 // 512
                pend = min(w0 + w, (q4 + 1) * 512)
                pieces[t][q4].append((i, p0 - 512 * q4, pend - p0, base + p0 - w0))
                p0 = pend
    return pieces


ATT_PIECES = _attn_pieces()

# GEMM1 column chunks: banded heads only (0-2 -> [0,384), 5-7 -> [640,1024))
G1_CHUNKS = ((0, 384), (640, 384))


def _build_program(phases=3):
    nc = bacc.Bacc("TRN2", target_bir_lowering=False, debug=False, num_devices=B)

    # w_in ships only the 6 banded heads' 6144 output columns, m-half-major:
    # [mh][kc/kt][(i)][384] so each m-half is one contiguous early DMA.
    if GEMM1_FP8:
        xt8 = nc.dram_tensor("xt8", [L, 2 * E], F8, kind="ExternalInput")
        w_in_hi = nc.dram_tensor("w_in_hi", [128, 6144], F8, kind="ExternalInput")
        w_in_lo = nc.dram_tensor("w_in_lo", [128, 6144], F8, kind="ExternalInput")
    else:
        xt = nc.dram_tensor("xt", [L, E], BF, kind="ExternalInput")
        w_in = nc.dram_tensor("w_in", [128, 6144], BF, kind="ExternalInput")
    if GEMM2_FP8:
        w_out_hi = nc.dram_tensor("w_out_hi", [128, NBH * E], F8, kind="ExternalInput")
        w_out_lo = nc.dram_tensor("w_out_lo", [128, NBH * E], F8, kind="ExternalInput")
    else:
        w_out = nc.dram_tensor("w_out", [128, 8 * E], BF, kind="ExternalInput")
    a_tab = nc.dram_tensor("a_tab", [128, 7 * WIN], BF, kind="ExternalInput")
    r34t = nc.dram_tensor("r34t", [128, 8], DT, kind="ExternalInput")
    outT = nc.dram_tensor("outT", [E, L], BF, kind="ExternalOutput")

    with tile.TileContext(nc) as tc:
        with (
            tc.tile_pool(name="const", bufs=1) as cpool,
            tc.tile_pool(name="vbuf", bufs=1) as vpool,
            tc.tile_pool(name="outp", bufs=4) as outpool,
            tc.tile_pool(name="ps8", bufs=8, space="PSUM") as ps8,
        ):
            if GEMM2_FP8:
                wo_hi_sb = cpool.tile([128, NBH * E], F8, tag="wo_hi_sb")
                wo_lo_sb = cpool.tile([128, NBH * E], F8, tag="wo_lo_sb")
                att_hi = cpool.tile([128, 2 * NBH * E], F8, tag="att_hi")
                att_lo = cpool.tile([128, 2 * NBH * E], F8, tag="att_lo")
            else:
                w_out_sb = cpool.tile([128, 8 * E], BF, tag="w_out_sb")
                att_sb = cpool.tile([128, 2 * NBH * E], BF, tag="att_sb")
            a_sb = cpool.tile([128, 7 * WIN], BF, tag="a_sb")
            r34_sb = cpool.tile([128, 8], DT, tag="r34_sb")
            scratch = cpool.tile([128, 512], BF, tag="scratch")
            v_sb = vpool.tile([128, NT * E], BF, tag="v_sb")

            # ---- PE warmup: ramp the p-state during the DMA fill.
            # One memset + uniform N=128 matmuls: a single dependency so the
            # PE never micro-gaps mid-warmup (any gap resets the ramp clock).
            nc.vector.memset(scratch[:, 0:128], 0.0)
            pw = ps8.tile([128, 512], DT, tag="ps")
            for _ in range(28):
                nc.tensor.matmul(
                    pw[:, 0:128], scratch[:, 0:128], scratch[:, 0:128],
                    start=True, stop=True,
                )

            # attention helpers -------------------------------------------
            copy_rr = [0]

            def emit_att_group(q4, pair=None):
                heads = list(enumerate(BANDED_HEADS))
                if pair is not None:
                    heads = heads[2 * pair : 2 * pair + 2]
                for bi, h in heads:
                    ms = ATT_PIECES[BI_TYPE[bi]][q4]
                    patt_t = ps8.tile([128, 512], DT, tag="ps")
                    patt = patt_t[:]
                    for n_, (i, col, wd, tcol) in enumerate(ms):
                        nc.tensor.matmul(
                            patt[:, col : col + wd],
                            v_sb[:, i * E + h * 128 : i * E + (h + 1) * 128],
                            a_sb[:, tcol : tcol + wd],
                            start=(n_ == 0),
                            stop=(n_ == len(ms) - 1),
                        )
                    s, qq = q4 // 2, q4 % 2
                    if GEMM2_FP8:
                        c, j = bi // 2, bi % 2
                        base = ((s * 3 + c) * 2 + j) * E + qq * 512
                        hi_dst = att_hi[:, base : base + 512]
                        lo_dst = att_lo[:, base : base + 512]
                        nc.scalar.activation(
                            hi_dst, patt,
                            mybir.ActivationFunctionType.Copy,
                            scale=ATT_SCALE,
                        )
                        nc.vector.scalar_tensor_tensor(
                            lo_dst, patt, ATT_SCALE, hi_dst,
                            mybir.AluOpType.mult, mybir.AluOpType.subtract,
                        )
                    else:
                        dst = att_sb[:, (s * NBH + bi) * E + qq * 512 :][:, :512]
                        if copy_rr[0] % 2 == 0:
                            nc.scalar.copy(dst, patt)
                        else:
                            nc.vector.tensor_copy(dst, patt)
                    copy_rr[0] += 1

            # ---- phase 1: v = x @ W_in.T (banded-head columns only) ----
            with (
                tc.tile_pool(name="w_in_p", bufs=1) as wpool,
                tc.tile_pool(name="xt_p", bufs=7) as xtpool,
            ):
                if GEMM1_FP8:
                    w_hi_sb = wpool.tile([128, 6144], F8, tag="w_hi_sb")
                    w_lo_sb = wpool.tile([128, 6144], F8, tag="w_lo_sb")

                    def load_xt(i, split=False):
                        t8 = xtpool.tile([128, 2 * E], F8, tag="xt")
                        if split:
                            # hi plane first: the hh/lh terms can start sooner
                            nc.sync.dma_start(
                                t8[:, 0:E], xt8[i * 128 : (i + 1) * 128, 0:E]
                            )
                            nc.sync.dma_start(
                                t8[:, E : 2 * E],
                                xt8[i * 128 : (i + 1) * 128, E : 2 * E],
                            )
                        else:
                            nc.sync.dma_start(t8[:], xt8[i * 128 : (i + 1) * 128, :])
                        return t8[:, 0:E], t8[:, E : 2 * E]

                    def load_w(mh):
                        nc.sync.dma_start(
                            w_hi_sb[:, mh * 3072 : (mh + 1) * 3072],
                            w_in_hi[:, mh * 3072 : (mh + 1) * 3072],
                        )
                        nc.sync.dma_start(
                            w_lo_sb[:, mh * 3072 : (mh + 1) * 3072],
                            w_in_lo[:, mh * 3072 : (mh + 1) * 3072],
                        )
                else:
                    w_in_sb = wpool.tile([128, 6144], BF, tag="w_in_sb")

                    def load_xt(i):
                        t_ = xtpool.tile([128, E], BF, tag="xt")
                        nc.sync.dma_start(t_[:], xt[i * 128 : (i + 1) * 128, :])
                        return t_

                    def load_w(mh):
                        nc.sync.dma_start(
                            w_in_sb[:, mh * 3072 : (mh + 1) * 3072],
                            w_in[:, mh * 3072 : (mh + 1) * 3072],
                        )

                # stream in consumption order; xt0 first (w's sem lands last)
                xts = [load_xt(0)]
                load_w(0)
                xts.append(load_xt(1))
                load_w(1)
                xts.append(load_xt(2))
                xts.append(load_xt(3))
                xts.append(load_xt(4))

                pvs = {}
                vsc = (1.0 / W_SCALE) if GEMM1_FP8 else 1.0

                def g1_chunk(i, mh):
                    m0, n = G1_CHUNKS[mh]
                    pvc = ps8.tile([128, 512], DT, tag="ps")
                    pvs[(i, mh)] = pvc
                    pv = pvc
                    if GEMM1_FP8:
                        th, tl = xts[i]
                        for term in range(3):
                            for kc in range(4):
                                sh = th[:, kc * 256 : (kc + 1) * 256].rearrange(
                                    "p (i l) -> p i l", i=2
                                )
                                sl = tl[:, kc * 256 : (kc + 1) * 256].rearrange(
                                    "p (i l) -> p i l", i=2
                                )
                                wh = w_hi_sb[
                                    :, (mh * 4 + kc) * 768 : (mh * 4 + kc + 1) * 768
                                ].rearrange("p (i m) -> p i m", i=2)
                                wl = w_lo_sb[
                                    :, (mh * 4 + kc) * 768 : (mh * 4 + kc + 1) * 768
                                ].rearrange("p (i m) -> p i m", i=2)
                                s_, m_ = ((sh, wh), (sl, wh), (sh, wl))[term]
                                nc.tensor.matmul(
                                    pv[:, 0:n],
                                    s_,
                                    m_,
                                    start=(kc == 0 and term == 0),
                                    stop=(kc == 3 and term == 2),
                                    perf_mode=mybir.MatmulPerfMode.DoubleRow,
                                )
                    else:
                        xt_t = xts[i]
                        for kt in range(8):
                            nc.tensor.matmul(
                                pv[:, 0:n],
                                xt_t[:, kt * 128 : (kt + 1) * 128],
                                w_in_sb[
                                    :, (mh * 8 + kt) * 384 : (mh * 8 + kt + 1) * 384
                                ],
                                start=(kt == 0),
                                stop=(kt == 7),
                            )

                def g1_copy(i, mh):
                    m0, n = G1_CHUNKS[mh]
                    pv = pvs.pop((i, mh))
                    dst = v_sb[:, i * E + m0 : i * E + m0 + n]
                    if (i + mh) % 2 == 0:
                        nc.scalar.activation(
                            dst, pv[:, 0:n],
                            mybir.ActivationFunctionType.Copy, scale=vsc,
                        )
                    else:
                        if GEMM1_FP8:
                            nc.vector.tensor_scalar_mul(dst, pv[:, 0:n], vsc)
                        else:
                            nc.vector.tensor_copy(dst, pv[:, 0:n])

                # tiles 0/1 interleave m-halves so PE work tracks DMA arrival
                for i_, mh_ in ((0, 0), (1, 0), (0, 1), (1, 1)):
                    g1_chunk(i_, mh_)
                    g1_copy(i_, mh_)

                xts.append(load_xt(5))
                nc.sync.dma_start(a_sb[:], a_tab[:])
                nc.sync.dma_start(r34_sb[:], r34t[:])

                for i in range(2, NT):
                    if i + 4 < NT:
                        xts.append(load_xt(i + 4))
                    if GEMM2_FP8:
                        if 4 <= i < 7:
                            c = i - 4
                            nc.sync.dma_start(
                                wo_hi_sb[:, c * 2048 : (c + 1) * 2048],
                                w_out_hi[:, c * 2048 : (c + 1) * 2048],
                            )
                            nc.sync.dma_start(
                                wo_lo_sb[:, c * 2048 : (c + 1) * 2048],
                                w_out_lo[:, c * 2048 : (c + 1) * 2048],
                            )
                    else:
                        if 4 <= i < 12:
                            c = i - 4
                            nc.sync.dma_start(
                                w_out_sb[:, c * 1024 : (c + 1) * 1024],
                                w_out[:, c * 1024 : (c + 1) * 1024],
                            )
                    for mh_ in (0, 1):
                        g1_chunk(i, mh_)
                    for mh_ in (0, 1):
                        g1_copy(i, mh_)
                    if i in ATT_EMIT_AT:
                        for q4_, pair_ in ATT_EMIT_AT[i]:
                            emit_att_group(q4_, pair_)

            emit_att_group(3, 2)

            # ---- phase 3: out^T = W_out @ att^T + r34 bias ----
            osc = 1.0 / (ATT_SCALE * W_SCALE)
            nout = 0
            for qc in range(4):
                s, qq = qc // 2, qc % 2
                for et in range(8):
                    po_t = ps8.tile([128, 512], DT, tag="ps")
                    po = po_t[:]
                    # final chunk splits so the very last output DMA is tiny;
                    # sub-chunk 2 gets its own PSUM bank so its matmuls don't
                    # serialize behind sub-chunk 1's PSUM read (bank tracker)
                    last = qc == 3 and et == 7
                    subs = ((0, 384), (384, 128)) if last else ((0, 512),)
                    for si, (s0, sn) in enumerate(subs):
                        if si == 1:
                            po_t2 = ps8.tile([128, 512], DT, tag="ps")
                            po = po_t2[:]
                        p0 = 0 if si == 1 else s0
                        if GEMM2_FP8:
                            nmm = 0
                            for c in range(3):
                                rh = att_hi[
                                    :, (s * 3 + c) * 2048 :][:, :2048].rearrange(
                                    "p (j q) -> p j q", j=2
                                )[:, :, qq * 512 + s0 : qq * 512 + s0 + sn]
                                rl = att_lo[
                                    :, (s * 3 + c) * 2048 :][:, :2048].rearrange(
                                    "p (j q) -> p j q", j=2
                                )[:, :, qq * 512 + s0 : qq * 512 + s0 + sn]
                                lh = wo_hi_sb[
                                    :, c * 2048 : (c + 1) * 2048
                                ].rearrange("p (j e) -> p j e", j=2)[
                                    :, :, et * 128 : (et + 1) * 128
                                ]
                                ll = wo_lo_sb[
                                    :, c * 2048 : (c + 1) * 2048
                                ].rearrange("p (j e) -> p j e", j=2)[
                                    :, :, et * 128 : (et + 1) * 128
                                ]
                                for s_, m_ in ((lh, rh), (ll, rh), (lh, rl)):
                                    nc.tensor.matmul(
                                        po[:, p0 : p0 + sn], s_, m_,
                                        start=(nmm == 0),
                                        stop=(nmm == 8),
                                        perf_mode=mybir.MatmulPerfMode.DoubleRow,
                                    )
                                    nmm += 1
                        else:
                            for bi, h in enumerate(BANDED_HEADS):
                                nc.tensor.matmul(
                                    po[:, p0 : p0 + sn],
                                    w_out_sb[
                                        :, h * E + et * 128 : h * E + (et + 1) * 128
                                    ],
                                    att_sb[
                                        :, (s * NBH + bi) * E + qq * 512 + s0 :
                                    ][:, :sn],
                                    start=(bi == 0),
                                    stop=(bi == NBH - 1),
                                )
                        ot = outpool.tile([128, 512], BF, tag="ot")
                        bias = r34_sb[:, et : et + 1]
                        use_act = (nout % 2 == 0) if not last else (si == 0)
                        if GEMM2_FP8:
                            if use_act:
                                nc.scalar.activation(
                                    ot[:, 0:sn], po[:, p0 : p0 + sn],
                                    mybir.ActivationFunctionType.Identity,
                                    bias=bias, scale=osc,
                                )
                            else:
                                nc.vector.tensor_scalar(
                                    ot[:, 0:sn], po[:, p0 : p0 + sn], osc, bias,
                                    mybir.AluOpType.mult, mybir.AluOpType.add,
                                )
                        else:
                            if use_act:
                                nc.scalar.add(ot[:, 0:sn], po[:, p0 : p0 + sn], bias)
                            else:
                                nc.vector.tensor_scalar_add(
                                    ot[:, 0:sn], po[:, p0 : p0 + sn], bias
                                )
                        nout += 1
                        dma_eng = nc.sync
                        if last and si == 0:
                            dma_eng = nc.gpsimd
                        dma_eng.dma_start(
                            outT[
                                et * 128 : (et + 1) * 128,
                                qc * 512 + s0 : qc * 512 + s0 + sn,
                            ],
                            ot[:, 0:sn],
                        )

    nc.compile()
    return nc


# ------------------------- host-side preparation ---------------------------

_NPBF = mybir.dt.np(BF)
_NPF8 = mybir.dt.np(F8)
# banded-head output columns of W_in^T, m-half-major (0:384 then 640:1024)
_MCOLS = np.concatenate([np.arange(0, 384), np.arange(640, 1024)])


def _host_wf_wl():
    """Exact 'first'/'last' head weight vectors over their 16-key support."""
    j = np.arange(L, dtype=np.float64)
    zf = _g(j - 0.0).sum()
    zl = _g(j - (L - 1.0)).sum()
    wf = _g(np.arange(16)) / zf
    wl = _g(np.arange(L - 16, L) - (L - 1.0)) / zl
    return wf, wl


def _host_r34(x, W_in, W_out):
    """[B, 128, 8] fp32: per-core output bias rows from the 'first'/'last'
    heads, computed exactly on the host (r34t[p, et] = r34[et*128 + p])."""
    wf, wl = _host_wf_wl()
    x64 = x.astype(np.float64)
    s3 = np.einsum("k,bke->be", wf, x64[:, 0:16, :])        # [B, E]
    s4 = np.einsum("k,bke->be", wl, x64[:, L - 16 : L, :])
    W_in64 = W_in.astype(np.float64)
    W_out64 = W_out.astype(np.float64)
    u3 = s3 @ W_in64.T[:, 384:512]                          # [B, 128]
    u4 = s4 @ W_in64.T[:, 512:640]
    r34 = u3 @ W_out64.T[384:512, :] + u4 @ W_out64.T[512:640, :]  # [B, E]
    return np.ascontiguousarray(
        r34.reshape(B, 8, 128).transpose(0, 2, 1)
    ).astype(np.float32)


def _pack_xt_bf16(x):
    # xt[b, i*128 + p, kt*128 + l] = x[b, i*128 + l, kt*128 + p]
    t = x.reshape(B, NT, 128, 8, 128).transpose(0, 1, 4, 3, 2)
    return np.ascontiguousarray(t).reshape(B * L, E).astype(_NPBF)


def _pack_xt_fp8(xq):
    # xt[b, i*128 + p, kc*256 + ipl*128 + l] = xq[b, i*128 + l, kc*256 + ipl*128 + p]
    t = xq.reshape(B, NT, 128, 4, 2, 128).transpose(0, 1, 5, 3, 4, 2)
    return np.ascontiguousarray(t).reshape(B * L, E)


def _pack_w_bf16(Wt):
    # w[p, (mh*8 + kt)*384 + m] = W.T[kt*128 + p, mcol(mh, m)]
    t = Wt.reshape(8, 128, E)[:, :, _MCOLS]          # [kt, p, mh*384+m]
    t = t.reshape(8, 128, 2, 384).transpose(1, 2, 0, 3)
    return np.ascontiguousarray(t).reshape(128, 6144).astype(_NPBF)


def _pack_w_fp8(Wq):
    # w[p, ((mh*4 + kc)*2 + ipl)*384 + m] = Wq[kc*256 + ipl*128 + p, mcol(mh, m)]
    t = Wq.reshape(4, 2, 128, E)[:, :, :, _MCOLS]    # [kc, ipl, p, mh*384+m]
    t = t.reshape(4, 2, 128, 2, 384).transpose(2, 3, 0, 1, 4)
    return np.ascontiguousarray(t).reshape(128, 6144)


def _pack_wo_fp8(Wq):
    # Wq: [NBH*128, E] rows = banded-head-major features (bi, p).
    # wo[p, c*2048 + j*1024 + e] = Wq[(c*2 + j)*128 + p, e]
    t = Wq.reshape(3, 2, 128, E).transpose(2, 0, 1, 3)
    return np.ascontiguousarray(t).reshape(128, NBH * E)


def _split_f8(a):
    hi = a.astype(_NPF8)
    lo = (a - hi.astype(np.float32)).astype(_NPF8)
    return hi, lo


class _Runner:
    """Builds the Bass program once and caches a jitted shard_map executable
    (one batch element per NeuronCore)."""

    def __init__(self):
        import jax
        from jax.sharding import Mesh, PartitionSpec
        from jax.experimental.shard_map import shard_map

        self.jax = jax
        _b2j.install_neuronx_cc_hook()
        nc = _build_program()
        self.nc = nc
        self.a_tab_np = _attn_tables().astype(_NPBF)

        partition_name = (
            nc.partition_id_tensor.name if nc.partition_id_tensor else None
        )
        in_names = []
        out_names = []
        out_avals = []
        for alloc in nc.m.functions[0].allocations:
            if not isinstance(alloc, mybir.MemoryLocationSet):
                continue
            name = alloc.memorylocations[0].name
            if alloc.kind == "ExternalInput":
                if name != partition_name:
                    in_names.append(name)
            elif alloc.kind == "ExternalOutput":
                out_names.append(name)
                out_avals.append(
                    jax.core.ShapedArray(
                        tuple(alloc.tensor_shape), mybir.dt.np(alloc.dtype)
                    )
                )
        self.in_names = in_names
        self.out_names = out_names
        self.out_avals = out_avals
        n_params = len(in_names)
        n_outs = len(out_names)
        all_names = tuple(in_names) + tuple(out_names)
        if partition_name is not None:
            all_names = all_names + (partition_name,)

        def _body(*args):
            operands = list(args)
            if partition_name is not None:
                operands.append(_b2j.partition_id_tensor())
            outs = _b2j._bass_exec_p.bind(
                *operands,
                out_avals=tuple(out_avals),
                in_names=all_names,
                out_names=tuple(out_names),
                lowering_input_output_aliases=(),
                sim_require_finite=True,
                sim_require_nnan=True,
                nc=nc,
            )
            return tuple(outs)

        devices = jax.devices()[:B]
        assert len(devices) == B
        self.mesh = Mesh(np.asarray(devices), ("core",))
        in_specs = (PartitionSpec("core"),) * (n_params + n_outs)
        out_specs = (PartitionSpec("core"),) * n_outs
        self.sharded = jax.jit(
            shard_map(
                _body,
                mesh=self.mesh,
                in_specs=in_specs,
                out_specs=out_specs,
                check_rep=False,
            ),
            donate_argnums=tuple(range(n_params, n_params + n_outs)),
            keep_unused=True,
        )

    def run_device(self, dev_args):
        jnp = self.jax.numpy
        zeros = [
            jnp.zeros((B * av.shape[0], *av.shape[1:]), av.dtype)
            for av in self.out_avals
        ]
        return self.sharded(*dev_args, *zeros)

    def prepare_inputs(self, x, W_in, W_out):
        jax = self.jax
        dev = {}
        if GEMM1_FP8:
            xh, xl = _split_f8(x)
            dev["xt8"] = np.concatenate(
                [_pack_xt_fp8(xh), _pack_xt_fp8(xl)], axis=1
            )
            Wt = np.ascontiguousarray(W_in.T) * np.float32(W_SCALE)
            Wh, Wl = _split_f8(Wt)
            dev["w_in_hi"] = np.concatenate([_pack_w_fp8(Wh)] * B, axis=0)
            dev["w_in_lo"] = np.concatenate([_pack_w_fp8(Wl)] * B, axis=0)
        else:
            dev["xt"] = _pack_xt_bf16(x)
            w_in_b = _pack_w_bf16(np.ascontiguousarray(W_in.T))
            dev["w_in"] = np.concatenate([w_in_b] * B, axis=0)
        if GEMM2_FP8:
            rows = np.concatenate(
                [np.arange(h * 128, (h + 1) * 128) for h in BANDED_HEADS]
            )
            Wq = np.ascontiguousarray(W_out.T[rows, :]) * np.float32(W_SCALE)
            Wh, Wl = _split_f8(Wq)
            dev["w_out_hi"] = np.concatenate([_pack_wo_fp8(Wh)] * B, axis=0)
            dev["w_out_lo"] = np.concatenate([_pack_wo_fp8(Wl)] * B, axis=0)
        else:
            w_out_b = _pack_w_bf16(np.ascontiguousarray(W_out.T))
            dev["w_out"] = np.concatenate([w_out_b] * B, axis=0)
        dev["a_tab"] = np.concatenate([self.a_tab_np] * B, axis=0)
        dev["r34t"] = _host_r34(x, W_in, W_out).reshape(B * 128, 8)
        return [jax.device_put(dev[name]) for name in self.in_names]

    def __call__(self, x, W_in, W_out):
        args = self.prepare_inputs(x, W_in, W_out)
        outs = self.run_device(args)
        outT = np.asarray(outs[self.out_names.index("outT")])
        # outT: [B*E, L] bf16 -> [B, L, E] fp32
        return np.ascontiguousarray(
            outT.reshape(B, E, L).transpose(0, 2, 1)
        ).astype(np.float32)


_CACHE = {}


def _get_runner() -> _Runner:
    if "runner" not in _CACHE:
        _CACHE["runner"] = _Runner()
    return _CACHE["runner"]


def kernel(x, W_in, W_out):
    x = np.ascontiguousarray(np.asarray(x, dtype=np.float32))
    W_in = np.ascontiguousarray(np.asarray(W_in, dtype=np.float32))
    W_out = np.ascontiguousarray(np.asarray(W_out, dtype=np.float32))
    assert x.shape == (B, L, E)
    return _get_runner()(x, W_in, W_out)


if __name__ == "__main__":
    rng = np.random.default_rng(0)
    x = rng.standard_normal((B, L, E), dtype=np.float32)
    W_in = rng.standard_normal((E, E), dtype=np.float32) * 0.05
    W_out = rng.standard_normal((E, E), dtype=np.float32) * 0.05
    y = kernel(x, W_in, W_out)
    print("out", y.shape, y.dtype, np.abs(y).mean())
